# revision 16
# baseline (speedup 1.0000x reference)
"""Trainium2 Bass kernel for DeTrAttention -- HEAD-SHARDED variant.

Full op: out = softmax((q@Wq+bq)(k@Wk+bk)^T / sqrt(64)) (v@Wv+bv) @ Wo + bo
Shapes: q,k,v [B=2, S=2048, H=1024], NH=16 heads, HD=64.

Sharding (8 cores): data-parallel over batch (2 groups of 4 cores); within a
group, core g owns HEADS 4g..4g+3 end-to-end (Q/K/V projected only into its
256 feature columns, attention over ALL 2048 tokens) -- so no K/V exchange
is needed at all.  The output projection contracts each core's 256 context
features against its 256-row slice of Wo, giving a PARTIAL output for all
2048 tokens; a 4-rank bf16 ReduceScatter sums the partials and hands core g
its 512 token rows.  Unlike the K/V AllGather of the query-sharded variant
(which sat ~60us un-hidden on the critical path), the ReduceScatter has a
full attention phase to complete in and moves half the bytes.

Schedule: software-pipelined like the query-sharded variant: iteration
it-1's deferred output projection + ReduceScatter launch, it+1's input
loads + Q/K/V projections, and it-1's reduced-output writeback all
interleave into iteration it's attention steps (paced evenly).  Q/K/V
projections write kpT/qpT/vp SBUF directly (no DRAM staging); all four
weight matrices stay SBUF-resident (loaded once).  Scores use PE row
tiling: each head's QK^T has K=64 contraction, so head pairs run as
[64,128] stationaries at tile_position (0,0)/(64,0) concurrently.

Precision: bf16 weights/activations (Wq,bq pre-scaled by 1/sqrt(64)), fp32
PSUM accumulation, bf16 partials into the ReduceScatter (adds ~1e-3 noise,
well inside the 2e-2 gate).  exp(s - 2) with the ones-column Z trick as
before.
"""

import numpy as np

import concourse.bass as bass
import concourse.tile as tile
from concourse import bacc, mybir
from concourse.bass_utils import run_bass_kernel_spmd

F32 = mybir.dt.float32
BF16 = mybir.dt.bfloat16
U8 = mybir.dt.uint8

B, S, H, NH = 2, 2048, 1024, 16
HD = H // NH  # 64
N_CORES = 8
CPG = 4            # cores per batch group
HL = NH // CPG     # local heads (4)
FB = HL * HD // 128  # local feature blocks (2)
SQ = S // CPG      # output token rows owned per core (512)
KB = H // 128      # contraction 128-blocks (8)
MB = H // 128      # output-feature 128-blocks (8)
KTB = S // 128     # key-token 128-blocks (16)
QC = S // 512      # query 512-chunks (4)
EBIAS = -2.0       # exp(s + EBIAS); cancels in softmax
GROUPS = [[0, 1, 2, 3], [4, 5, 6, 7]]


def build_nc(sreps=1, upto=3, sim=False):
    """Per-core Bass program (SPMD, identical on all 8 cores).

    sim=True replaces the ReduceScatter with a local DMA copy (timing
    shape only; numerically wrong) so single-core TimelineSim works.
    """
    nc = bacc.Bacc("TRN2", target_bir_lowering=False, debug=False,
                   num_devices=8)

    qT = nc.dram_tensor("qT", [H, S], BF16, kind="ExternalInput").ap()
    kT = nc.dram_tensor("kT", [H, S], BF16, kind="ExternalInput").ap()
    vT = nc.dram_tensor("vT", [H, S], BF16, kind="ExternalInput").ap()
    Wq = nc.dram_tensor("Wq", [H, HL * HD], BF16, kind="ExternalInput").ap()
    Wk = nc.dram_tensor("Wk", [H, HL * HD], BF16, kind="ExternalInput").ap()
    Wv = nc.dram_tensor("Wv", [H, HL * HD], BF16, kind="ExternalInput").ap()
    Wo = nc.dram_tensor("Wo", [HL * HD, H], BF16, kind="ExternalInput").ap()
    bqT = nc.dram_tensor("bqT", [128, FB], F32, kind="ExternalInput").ap()
    bkT = nc.dram_tensor("bkT", [128, FB], F32, kind="ExternalInput").ap()
    boT = nc.dram_tensor("boT", [128, MB], F32, kind="ExternalInput").ap()
    # bv pre-broadcast to all partitions, [128, HL, HD] (no ones col needed;
    # the ones column of vp is memset once)
    bvp = nc.dram_tensor("bvp", [128, HL, HD], BF16,
                         kind="ExternalInput").ap()
    outT = nc.dram_tensor("outT", [H, SQ], BF16, kind="ExternalOutput").ap()

    qT_p = qT.rearrange("(kb p) t -> p kb t", p=128)
    kT_p = kT.rearrange("(kb p) t -> p kb t", p=128)
    vT_p = vT.rearrange("(kb p) t -> p kb t", p=128)
    Wq_p = Wq.rearrange("(kb p) o -> p kb o", p=128)
    Wk_p = Wk.rearrange("(kb p) o -> p kb o", p=128)
    Wv_p = Wv.rearrange("(kb p) o -> p kb o", p=128)
    Wo_p = Wo.rearrange("(fb p) o -> p fb o", p=128)
    outT_p = outT.rearrange("(ob p) t -> p ob t", p=128)

    with tile.TileContext(nc) as tc:
        with tc.tile_pool(name="consts", bufs=1) as consts, \
             tc.tile_pool(name="persist", bufs=1) as persist, \
             tc.tile_pool(name="stream", bufs=2) as stream, \
             tc.tile_pool(name="exps", bufs=3) as exps, \
             tc.tile_pool(name="stg", bufs=2) as stgp, \
             tc.tile_pool(name="ow", bufs=2) as owp, \
             tc.tile_pool(name="dramp", bufs=2, space="DRAM") as dramp, \
             tc.tile_pool(name="ps2b", bufs=2, space="PSUM") as ps2b, \
             tc.tile_pool(name="psa", bufs=2, space="PSUM") as psA, \
             tc.tile_pool(name="psacc", bufs=2, space="PSUM") as psacc:

            ebias = consts.tile([128, 1], F32)
            nc.vector.memset(ebias, EBIAS)
            bq_sb = consts.tile([128, FB], F32, tag="bq")
            bk_sb = consts.tile([128, FB], F32, tag="bk")
            bo_sb = consts.tile([128, MB], F32, tag="bo")
            bvp_sb = consts.tile([128, HL, HD], BF16, tag="bvp")
            nc.sync.dma_start(out=bq_sb, in_=bqT)
            nc.sync.dma_start(out=bk_sb, in_=bkT)
            nc.sync.dma_start(out=bo_sb, in_=boT)
            nc.sync.dma_start(out=bvp_sb, in_=bvp)
            # resident weights (loaded once)
            wq_sb = consts.tile([128, KB, HL * HD], BF16, tag="wq")
            wk_sb = consts.tile([128, KB, HL * HD], BF16, tag="wk")
            wv_sb = consts.tile([128, KB, HL * HD], BF16, tag="wv")
            wo_sb = consts.tile([128, FB, H], BF16, tag="wo")
            nc.sync.dma_start(out=wq_sb, in_=Wq_p)
            nc.sync.dma_start(out=wk_sb, in_=Wk_p)
            nc.sync.dma_start(out=wv_sb, in_=Wv_p)
            nc.sync.dma_start(out=wo_sb, in_=Wo_p)

            # double-buffered per-iteration state (parity = iteration % 2)
            kpTs = [persist.tile([128, FB, S], BF16, tag=f"kpT{x}",
                                 name=f"kpT{x}") for x in "AB"]
            qpTs = [persist.tile([128, FB, S], BF16, tag=f"qpT{x}",
                                 name=f"qpT{x}") for x in "AB"]
            vps = [persist.tile([128, KTB, HL, HD + 1], BF16, tag=f"vp{x}",
                                name=f"vp{x}") for x in "AB"]
            ctxnTs = [persist.tile([128, FB, S], BF16, tag=f"ctxnT{x}",
                                   name=f"ctxnT{x}") for x in "AB"]
            # ones column of vp written once; projections only touch [0:HD]
            for _v in vps:
                nc.vector.memset(_v[:, :, :, HD:HD + 1], 1.0)

            def emit_iter_inputs(it):
                """Unit closures for iteration `it`'s input pipeline:
                input loads + K/Q/V projections straight into SBUF."""
                par = it % 2
                st8 = {}
                ems = []

                def e_in(name, dram, half):
                    def u():
                        if half == 0:
                            t = stream.tile([128, KB, S], BF16, tag="in3",
                                            name=f"{name}_t")
                            st8[name] = t
                        nc.sync.dma_start(
                            out=st8[name][:, :, half * 1024:(half + 1) * 1024],
                            in_=dram[:, :, half * 1024:(half + 1) * 1024])
                    return u

                def e_kq(which, w_sb, dst, b_sb, fb, tc4):
                    def u():
                        ps = psA.tile([128, 512], F32, tag="psa",
                                      name=f"{which}p{fb}{tc4}")
                        src = st8[which]
                        for kb in range(KB):
                            nc.tensor.matmul(
                                ps, w_sb[:, kb, fb * 128:(fb + 1) * 128],
                                src[:, kb, tc4 * 512:(tc4 + 1) * 512],
                                start=(kb == 0), stop=(kb == KB - 1))
                        nc.vector.tensor_scalar_add(
                            dst[:, fb, tc4 * 512:(tc4 + 1) * 512], ps,
                            b_sb[:, fb:fb + 1])
                    return u

                def e_vp(st):
                    def u():
                        ps = psA.tile([128, HL * HD], F32, tag="psa",
                                      name=f"vp{st}")
                        for kb in range(KB):
                            nc.tensor.matmul(
                                ps, st8["v"][:, kb, st * 128:(st + 1) * 128],
                                wv_sb[:, kb, :],
                                start=(kb == 0), stop=(kb == KB - 1))
                        nc.vector.tensor_add(
                            vps[par][:, st, :, 0:HD],
                            ps.rearrange("p (hh d) -> p hh d", d=HD),
                            bvp_sb)
                    return u

                for half in range(2):
                    ems.append(e_in("k", kT_p, half))
                for fb in range(FB):
                    for tc4 in range(QC):
                        ems.append(e_kq("k", wk_sb, kpTs[par], bk_sb,
                                        fb, tc4))
                for half in range(2):
                    ems.append(e_in("v", vT_p, half))
                for st in range(KTB):
                    ems.append(e_vp(st))
                for half in range(2):
                    ems.append(e_in("q", qT_p, half))
                for fb in range(FB):
                    for tc4 in range(QC):
                        ems.append(e_kq("q", wq_sb, qpTs[par], bq_sb,
                                        fb, tc4))
                return ems, st8

            def mk_carry(par):
                """Deferred tail of iteration with parity `par`: output
                projection (staged dest-major to DRAM), ReduceScatter
                launch, and (last) the reduced-output writeback."""
                rs_in = dramp.tile([CPG, 128, MB, SQ], BF16, tag="rsi",
                                   name="rs_in")
                rs_out = dramp.tile([128, MB, SQ], BF16, tag="rso",
                                    name="rs_out")
                units = []

                def u_ob(ob):
                    for tc4 in range(QC):
                        po = psA.tile([128, 512], F32, tag="psa",
                                      name=f"po{ob}{tc4}")
                        for fb in range(FB):
                            nc.tensor.matmul(
                                po, wo_sb[:, fb, ob * 128:(ob + 1) * 128],
                                ctxnTs[par][:, fb,
                                            tc4 * 512:(tc4 + 1) * 512],
                                start=(fb == 0), stop=(fb == FB - 1))
                        pst = stgp.tile([128, SQ], BF16, tag="kst",
                                        name="pst")
                        nc.vector.tensor_copy(pst, po)
                        nc.sync.dma_start(out=rs_in[tc4, :, ob, :], in_=pst)
                for ob in range(MB):
                    units.append(lambda ob=ob: u_ob(ob))

                def u_rs():
                    if sim:
                        nc.sync.dma_start(out=rs_out, in_=rs_in[0])
                    else:
                        nc.gpsimd.collective_compute(
                            "ReduceScatter", mybir.AluOpType.add,
                            ins=[rs_in.opt()], outs=[rs_out.opt()],
                            replica_groups=GROUPS)
                units.append(u_rs)

                def u_write():
                    ot = owp.tile([128, MB, SQ], BF16, tag="ot", name="ot")
                    nc.gpsimd.dma_start(out=ot, in_=rs_out)
                    for ob in range(MB):
                        oo = stgp.tile([128, SQ], BF16, tag="kst",
                                       name="oo")
                        nc.vector.tensor_scalar_add(oo, ot[:, ob, :],
                                                    bo_sb[:, ob:ob + 1])
                        nc.sync.dma_start(out=outT_p[:, ob, :], in_=oo)
                return units, u_write

            pending, pstate = emit_iter_inputs(0)
            carry = []        # prev iteration's outproj + RS units
            carry_tail = None  # prev iteration's writeback unit
            for it in range(sreps):
                par = it % 2
                kpT, vp, qpT, ctxnT = (kpTs[par], vps[par], qpTs[par],
                                       ctxnTs[par])
                for e in pending:
                    e()
                pending = []

                front = []
                if it + 1 < sreps:
                    front, pstate = emit_iter_inputs(it + 1)
                pending = front  # for the non-attention paths

                if upto < 2:
                    nc.sync.dma_start(out=outT_p[:, 0, :],
                                      in_=kpT[:, 0, 0:SQ])
                    nc.sync.dma_start(out=outT_p[:, 1, :],
                                      in_=qpT[:, 0, 0:SQ])
                    nc.gpsimd.dma_start(out=outT_p[:, 2, 0:65],
                                        in_=vp[:, 0, 0, :])
                    carry, carry_tail = [], None
                    continue

                # ---- attention: 128 steps of (row-tiled scores pair, exp,
                # lag-1 ctx pair).  carry units (it-1's outproj + RS
                # launch) go 1-per-step so the ReduceScatter starts as
                # early as possible; front units (it+1's inputs +
                # projections) spread over the remaining steps; the
                # writeback of it-1 goes at the tail ----
                units = carry + front
                n_steps = FB * QC * KTB
                nfast = len(carry)
                ustep = (len(units) - nfast) / max(1, n_steps - nfast)
                uacc = 0.0
                ui = 0
                stepn = 0
                for hb in range(FB):
                    for qc in range(QC):
                        accs = [psacc.tile([128, 512], F32, tag="acc",
                                           name=f"acc{j}") for j in range(2)]
                        prev = None
                        for ktb in range(KTB):
                            p1 = ps2b.tile([128, 2, 512], F32, tag="sc",
                                           name="sc")
                            for j in range(2):
                                nc.tensor.matmul(
                                    p1[:, j, :],
                                    kpT[64 * j:64 * j + 64, hb,
                                        ktb * 128:(ktb + 1) * 128],
                                    qpT[64 * j:64 * j + 64, hb,
                                        qc * 512:(qc + 1) * 512],
                                    start=True, stop=True)
                            et = exps.tile([128, 2, 512], BF16, tag="et",
                                           name="et")
                            nc.scalar.activation(
                                out=et, in_=p1,
                                func=mybir.ActivationFunctionType.Exp,
                                bias=ebias[:, 0:1])
                            if prev is not None:
                                pk, pet = prev
                                for j in range(2):
                                    nc.tensor.matmul(
                                        accs[j][0:HD + 1, :],
                                        vp[:, pk, 2 * hb + j, :],
                                        pet[:, j, :],
                                        start=(pk == 0), stop=(pk == KTB - 1))
                            prev = (ktb, et)
                            stepn += 1
                            uacc += 1.0 if stepn <= nfast else ustep
                            while ui < len(units) and uacc >= ui + 1:
                                units[ui]()
                                ui += 1
                        pk, pet = prev
                        for j in range(2):
                            nc.tensor.matmul(
                                accs[j][0:HD + 1, :],
                                vp[:, pk, 2 * hb + j, :], pet[:, j, :],
                                start=(pk == 0), stop=(pk == KTB - 1))
                        for j in range(2):
                            zr = stgp.tile([1, 512], BF16, tag="zr",
                                           name="zr")
                            with nc.allow_low_precision(
                                    reason="softmax 1/Z; DVE mul"):
                                nc.vector.reciprocal(zr,
                                                     accs[j][HD:HD + 1, :])
                            zb = stgp.tile([64, 512], BF16, tag="zb",
                                           name="zb")
                            nc.gpsimd.partition_broadcast(zb, zr)
                            nc.vector.tensor_mul(
                                ctxnT[64 * j:64 * j + 64, hb,
                                      qc * 512:(qc + 1) * 512],
                                accs[j][0:HD, :], zb)
                while ui < len(units):
                    units[ui]()
                    ui += 1
                if carry_tail is not None:
                    carry_tail()
                pending = []

                if upto < 3:
                    nc.sync.dma_start(out=outT_p[:, 0, :],
                                      in_=ctxnT[:, 0, 0:SQ])
                    carry, carry_tail = [], None
                    continue

                carry, carry_tail = mk_carry(par)
                if it == sreps - 1:
                    for u in carry:
                        u()
                    carry_tail()
                    carry, carry_tail = [], None

    nc.compile()
    return nc


def shard_inputs(q, k, v, Wq, bq, Wk, bk, Wv, bv, Wo, bo):
    """Host-side sharding: per-core input dicts (head-sharded)."""
    import ml_dtypes
    bf16 = ml_dtypes.bfloat16
    scale = np.float32(1.0 / np.sqrt(HD))
    c32 = lambda a: np.ascontiguousarray(a, dtype=np.float32)
    cbf = lambda a: np.ascontiguousarray(np.asarray(a, dtype=np.float32),
                                         dtype=bf16)
    Wq32, Wk32, Wv32, Wo32 = c32(Wq) * scale, c32(Wk), c32(Wv), c32(Wo)
    bq32, bk32, bv32 = c32(bq) * scale, c32(bk), c32(bv)
    boT = np.ascontiguousarray(c32(bo).reshape(MB, 128).T)
    qT = [cbf(np.asarray(q[b], np.float32).T) for b in range(B)]
    kT = [cbf(np.asarray(k[b], np.float32).T) for b in range(B)]
    vT = [cbf(np.asarray(v[b], np.float32).T) for b in range(B)]
    in_maps = []
    for c in range(N_CORES):
        b, g = c // CPG, c % CPG
        fsl = slice(g * HL * HD, (g + 1) * HL * HD)
        bvp = np.broadcast_to(
            cbf(bv32[fsl]).reshape(1, HL, HD), (128, HL, HD))
        in_maps.append({
            "qT": qT[b], "kT": kT[b], "vT": vT[b],
            "Wq": cbf(Wq32[:, fsl]), "Wk": cbf(Wk32[:, fsl]),
            "Wv": cbf(Wv32[:, fsl]),
            "Wo": cbf(Wo32[fsl, :]),
            "bqT": np.ascontiguousarray(bq32[fsl].reshape(FB, 128).T),
            "bkT": np.ascontiguousarray(bk32[fsl].reshape(FB, 128).T),
            "boT": boT,
            "bvp": np.ascontiguousarray(bvp),
        })
    return in_maps


_NC_CACHE = {}


def get_nc():
    if "nc" not in _NC_CACHE:
        _NC_CACHE["nc"] = build_nc()
    return _NC_CACHE["nc"]


def kernel(q, k, v, Wq, bq, Wk, bk, Wv, bv, Wo, bo):
    q, k, v = np.asarray(q), np.asarray(k), np.asarray(v)
    in_maps = shard_inputs(q, k, v, Wq, bq, Wk, bk, Wv, bv, Wo, bo)
    nc = get_nc()
    res = run_bass_kernel_spmd(nc, in_maps, core_ids=list(range(N_CORES)))
    out = np.empty((B, S, H), dtype=np.float32)
    for c in range(N_CORES):
        b, r0 = c // CPG, (c % CPG) * SQ
        out[b, r0:r0 + SQ, :] = np.asarray(
            res.results[c]["outT"], dtype=np.float32).T
    return out


# revision 19
# speedup vs baseline: 1.0205x; 1.0205x over previous
"""Trainium2 Bass kernel for DeTrAttention -- HEAD-SHARDED variant.

Full op: out = softmax((q@Wq+bq)(k@Wk+bk)^T / sqrt(64)) (v@Wv+bv) @ Wo + bo
Shapes: q,k,v [B=2, S=2048, H=1024], NH=16 heads, HD=64.

Sharding (8 cores): data-parallel over batch (2 groups of 4 cores); within a
group, core g owns HEADS 4g..4g+3 end-to-end (Q/K/V projected only into its
256 feature columns, attention over ALL 2048 tokens) -- so no K/V exchange
is needed at all.  The output projection contracts each core's 256 context
features against its 256-row slice of Wo, giving a PARTIAL output for all
2048 tokens; a 4-rank bf16 ReduceScatter sums the partials and hands core g
its 512 token rows.  Unlike the K/V AllGather of the query-sharded variant
(which sat ~60us un-hidden on the critical path), the ReduceScatter has a
full attention phase to complete in and moves half the bytes.

Schedule: software-pipelined like the query-sharded variant: iteration
it-1's deferred output projection + ReduceScatter launch, it+1's input
loads + Q/K/V projections, and it-1's reduced-output writeback all
interleave into iteration it's attention steps (paced evenly).  Q/K/V
projections write kpT/qpT/vp SBUF directly (no DRAM staging); all four
weight matrices stay SBUF-resident (loaded once).  Scores use PE row
tiling: each head's QK^T has K=64 contraction, so head pairs run as
[64,128] stationaries at tile_position (0,0)/(64,0) concurrently.

Precision: bf16 weights/activations (Wq,bq pre-scaled by 1/sqrt(64)), fp32
PSUM accumulation, bf16 partials into the ReduceScatter (adds ~1e-3 noise,
well inside the 2e-2 gate).  exp(s - 2) with the ones-column Z trick as
before.
"""

import numpy as np

import concourse.bass as bass
import concourse.tile as tile
from concourse import bacc, mybir
from concourse.bass_utils import run_bass_kernel_spmd

F32 = mybir.dt.float32
BF16 = mybir.dt.bfloat16
U8 = mybir.dt.uint8

B, S, H, NH = 2, 2048, 1024, 16
HD = H // NH  # 64
N_CORES = 8
CPG = 4            # cores per batch group
HL = NH // CPG     # local heads (4)
FB = HL * HD // 128  # local feature blocks (2)
SQ = S // CPG      # output token rows owned per core (512)
KB = H // 128      # contraction 128-blocks (8)
MB = H // 128      # output-feature 128-blocks (8)
KTB = S // 128     # key-token 128-blocks (16)
QC = S // 512      # query 512-chunks (4)
EBIAS = -2.0       # exp(s + EBIAS); cancels in softmax
GROUPS = [[0, 1, 2, 3], [4, 5, 6, 7]]


def build_nc(sreps=1, upto=3, sim=False):
    """Per-core Bass program (SPMD, identical on all 8 cores).

    sim=True replaces the ReduceScatter with a local DMA copy (timing
    shape only; numerically wrong) so single-core TimelineSim works.
    """
    nc = bacc.Bacc("TRN2", target_bir_lowering=False, debug=False,
                   num_devices=8)

    qT = nc.dram_tensor("qT", [H, S], BF16, kind="ExternalInput").ap()
    kT = nc.dram_tensor("kT", [H, S], BF16, kind="ExternalInput").ap()
    vT = nc.dram_tensor("vT", [H, S], BF16, kind="ExternalInput").ap()
    Wq = nc.dram_tensor("Wq", [H, HL * HD], BF16, kind="ExternalInput").ap()
    Wk = nc.dram_tensor("Wk", [H, HL * HD], BF16, kind="ExternalInput").ap()
    Wv = nc.dram_tensor("Wv", [H, HL * HD], BF16, kind="ExternalInput").ap()
    Wo = nc.dram_tensor("Wo", [HL * HD, H], BF16, kind="ExternalInput").ap()
    bqT = nc.dram_tensor("bqT", [128, FB], F32, kind="ExternalInput").ap()
    bkT = nc.dram_tensor("bkT", [128, FB], F32, kind="ExternalInput").ap()
    boT = nc.dram_tensor("boT", [128, MB], F32, kind="ExternalInput").ap()
    # bv pre-broadcast to all partitions, [128, HL, HD] (no ones col needed;
    # the ones column of vp is memset once)
    bvp = nc.dram_tensor("bvp", [128, HL, HD], BF16,
                         kind="ExternalInput").ap()
    outT = nc.dram_tensor("outT", [H, SQ], BF16, kind="ExternalOutput").ap()

    qT_p = qT.rearrange("(kb p) t -> p kb t", p=128)
    kT_p = kT.rearrange("(kb p) t -> p kb t", p=128)
    vT_p = vT.rearrange("(kb p) t -> p kb t", p=128)
    Wq_p = Wq.rearrange("(kb p) o -> p kb o", p=128)
    Wk_p = Wk.rearrange("(kb p) o -> p kb o", p=128)
    Wv_p = Wv.rearrange("(kb p) o -> p kb o", p=128)
    Wo_p = Wo.rearrange("(fb p) o -> p fb o", p=128)
    outT_p = outT.rearrange("(ob p) t -> p ob t", p=128)

    with tile.TileContext(nc) as tc:
        with tc.tile_pool(name="consts", bufs=1) as consts, \
             tc.tile_pool(name="persist", bufs=1) as persist, \
             tc.tile_pool(name="stream", bufs=2) as stream, \
             tc.tile_pool(name="exps", bufs=3) as exps, \
             tc.tile_pool(name="stg", bufs=2) as stgp, \
             tc.tile_pool(name="ow", bufs=2) as owp, \
             tc.tile_pool(name="dramp", bufs=2, space="DRAM") as dramp, \
             tc.tile_pool(name="ps2b", bufs=2, space="PSUM") as ps2b, \
             tc.tile_pool(name="psa", bufs=2, space="PSUM") as psA, \
             tc.tile_pool(name="psacc", bufs=2, space="PSUM") as psacc:

            ebias = consts.tile([128, 1], F32)
            nc.vector.memset(ebias, EBIAS)
            bq_sb = consts.tile([128, FB], F32, tag="bq")
            bk_sb = consts.tile([128, FB], F32, tag="bk")
            bo_sb = consts.tile([128, MB], F32, tag="bo")
            bvp_sb = consts.tile([128, HL, HD], BF16, tag="bvp")
            nc.sync.dma_start(out=bq_sb, in_=bqT)
            nc.sync.dma_start(out=bk_sb, in_=bkT)
            nc.sync.dma_start(out=bo_sb, in_=boT)
            nc.sync.dma_start(out=bvp_sb, in_=bvp)
            # resident weights (loaded once)
            wq_sb = consts.tile([128, KB, HL * HD], BF16, tag="wq")
            wk_sb = consts.tile([128, KB, HL * HD], BF16, tag="wk")
            wv_sb = consts.tile([128, KB, HL * HD], BF16, tag="wv")
            wo_sb = consts.tile([128, FB, H], BF16, tag="wo")
            nc.sync.dma_start(out=wq_sb, in_=Wq_p)
            nc.sync.dma_start(out=wk_sb, in_=Wk_p)
            nc.sync.dma_start(out=wv_sb, in_=Wv_p)
            nc.sync.dma_start(out=wo_sb, in_=Wo_p)

            # double-buffered per-iteration state (parity = iteration % 2)
            kpTs = [persist.tile([128, FB, S], BF16, tag=f"kpT{x}",
                                 name=f"kpT{x}") for x in "AB"]
            qpTs = [persist.tile([128, FB, S], BF16, tag=f"qpT{x}",
                                 name=f"qpT{x}") for x in "AB"]
            vps = [persist.tile([128, KTB, HL, HD + 1], BF16, tag=f"vp{x}",
                                name=f"vp{x}") for x in "AB"]
            ctxnTs = [persist.tile([128, FB, S], BF16, tag=f"ctxnT{x}",
                                   name=f"ctxnT{x}") for x in "AB"]
            # ones column of vp written once; projections only touch [0:HD]
            for _v in vps:
                nc.vector.memset(_v[:, :, :, HD:HD + 1], 1.0)

            def emit_iter_inputs(it):
                """Unit closures for iteration `it`'s input pipeline:
                input loads + K/Q/V projections straight into SBUF."""
                par = it % 2
                st8 = {}
                ems = []

                def e_in(name, dram, half):
                    def u():
                        if half == 0:
                            t = stream.tile([128, KB, S], BF16, tag="in3",
                                            name=f"{name}_t")
                            st8[name] = t
                        nc.sync.dma_start(
                            out=st8[name][:, :, half * 1024:(half + 1) * 1024],
                            in_=dram[:, :, half * 1024:(half + 1) * 1024])
                    return u

                def e_kq(which, w_sb, dst, b_sb, fb, tc4):
                    def u():
                        ps = psA.tile([128, 512], F32, tag="psa",
                                      name=f"{which}p{fb}{tc4}")
                        src = st8[which]
                        for kb in range(KB):
                            nc.tensor.matmul(
                                ps, w_sb[:, kb, fb * 128:(fb + 1) * 128],
                                src[:, kb, tc4 * 512:(tc4 + 1) * 512],
                                start=(kb == 0), stop=(kb == KB - 1))
                        nc.vector.tensor_scalar_add(
                            dst[:, fb, tc4 * 512:(tc4 + 1) * 512], ps,
                            b_sb[:, fb:fb + 1])
                    return u

                def e_vp(st):
                    def u():
                        ps = psA.tile([128, HL * HD], F32, tag="psa",
                                      name=f"vp{st}")
                        for kb in range(KB):
                            nc.tensor.matmul(
                                ps, st8["v"][:, kb, st * 128:(st + 1) * 128],
                                wv_sb[:, kb, :],
                                start=(kb == 0), stop=(kb == KB - 1))
                        nc.vector.tensor_add(
                            vps[par][:, st, :, 0:HD],
                            ps.rearrange("p (hh d) -> p hh d", d=HD),
                            bvp_sb)
                    return u

                for half in range(2):
                    ems.append(e_in("k", kT_p, half))
                for fb in range(FB):
                    for tc4 in range(QC):
                        ems.append(e_kq("k", wk_sb, kpTs[par], bk_sb,
                                        fb, tc4))
                for half in range(2):
                    ems.append(e_in("v", vT_p, half))
                for st in range(KTB):
                    ems.append(e_vp(st))
                for half in range(2):
                    ems.append(e_in("q", qT_p, half))
                for fb in range(FB):
                    for tc4 in range(QC):
                        ems.append(e_kq("q", wq_sb, qpTs[par], bq_sb,
                                        fb, tc4))
                return ems, st8

            def mk_carry(par):
                """Deferred tail of iteration with parity `par`: output
                projection (staged dest-major to DRAM), ReduceScatter
                launch, and (last) the reduced-output writeback."""
                rs_in = dramp.tile([CPG, 128, MB, SQ], BF16, tag="rsi",
                                   name="rs_in")
                rs_out = dramp.tile([128, MB, SQ], BF16, tag="rso",
                                    name="rs_out")
                units = []

                def u_ob(ob):
                    for tc4 in range(QC):
                        po = psA.tile([128, 512], F32, tag="psa",
                                      name=f"po{ob}{tc4}")
                        for fb in range(FB):
                            nc.tensor.matmul(
                                po, wo_sb[:, fb, ob * 128:(ob + 1) * 128],
                                ctxnTs[par][:, fb,
                                            tc4 * 512:(tc4 + 1) * 512],
                                start=(fb == 0), stop=(fb == FB - 1))
                        pst = stgp.tile([128, SQ], BF16, tag="kst",
                                        name="pst")
                        nc.vector.tensor_copy(pst, po)
                        nc.sync.dma_start(out=rs_in[tc4, :, ob, :], in_=pst)
                for ob in range(MB):
                    units.append(lambda ob=ob: u_ob(ob))

                def u_rs():
                    if sim:
                        nc.sync.dma_start(out=rs_out, in_=rs_in[0])
                    else:
                        nc.gpsimd.collective_compute(
                            "ReduceScatter", mybir.AluOpType.add,
                            ins=[rs_in.opt()], outs=[rs_out.opt()],
                            replica_groups=GROUPS)
                units.append(u_rs)

                def u_write():
                    # SP queue, NOT gpsimd: a wait on the ReduceScatter at
                    # the head of the gpsimd FIFO would block the next
                    # attention's norm partition_broadcasts behind it
                    # (input loads queued on SP have far more slack).
                    ot = owp.tile([128, MB, SQ], BF16, tag="ot", name="ot")
                    nc.sync.dma_start(out=ot, in_=rs_out)
                    for ob in range(MB):
                        oo = stgp.tile([128, SQ], BF16, tag="kst",
                                       name="oo")
                        nc.vector.tensor_scalar_add(oo, ot[:, ob, :],
                                                    bo_sb[:, ob:ob + 1])
                        nc.sync.dma_start(out=outT_p[:, ob, :], in_=oo)
                return units, u_write

            pending, pstate = emit_iter_inputs(0)
            carry = []        # prev iteration's outproj + RS units
            carry_tail = None  # prev iteration's writeback unit
            for it in range(sreps):
                par = it % 2
                kpT, vp, qpT, ctxnT = (kpTs[par], vps[par], qpTs[par],
                                       ctxnTs[par])
                for e in pending:
                    e()
                pending = []

                front = []
                if it + 1 < sreps:
                    front, pstate = emit_iter_inputs(it + 1)
                pending = front  # for the non-attention paths

                if upto < 2:
                    nc.sync.dma_start(out=outT_p[:, 0, :],
                                      in_=kpT[:, 0, 0:SQ])
                    nc.sync.dma_start(out=outT_p[:, 1, :],
                                      in_=qpT[:, 0, 0:SQ])
                    nc.gpsimd.dma_start(out=outT_p[:, 2, 0:65],
                                        in_=vp[:, 0, 0, :])
                    carry, carry_tail = [], None
                    continue

                # ---- attention: 128 steps of (row-tiled scores pair, exp,
                # lag-1 ctx pair).  carry units (it-1's outproj + RS
                # launch) go 1-per-step so the ReduceScatter starts as
                # early as possible; front units (it+1's inputs +
                # projections) spread over the remaining steps; the
                # writeback of it-1 goes at the tail ----
                units = carry + front
                n_steps = FB * QC * KTB
                ustep = len(units) / n_steps
                uacc = 0.0
                ui = 0
                for hb in range(FB):
                    for qc in range(QC):
                        accs = [psacc.tile([128, 512], F32, tag="acc",
                                           name=f"acc{j}") for j in range(2)]
                        prev = None
                        for ktb in range(KTB):
                            p1 = ps2b.tile([128, 2, 512], F32, tag="sc",
                                           name="sc")
                            for j in range(2):
                                nc.tensor.matmul(
                                    p1[:, j, :],
                                    kpT[64 * j:64 * j + 64, hb,
                                        ktb * 128:(ktb + 1) * 128],
                                    qpT[64 * j:64 * j + 64, hb,
                                        qc * 512:(qc + 1) * 512],
                                    start=True, stop=True)
                            et = exps.tile([128, 2, 512], BF16, tag="et",
                                           name="et")
                            nc.scalar.activation(
                                out=et, in_=p1,
                                func=mybir.ActivationFunctionType.Exp,
                                bias=ebias[:, 0:1])
                            if prev is not None:
                                pk, pet = prev
                                for j in range(2):
                                    nc.tensor.matmul(
                                        accs[j][0:HD + 1, :],
                                        vp[:, pk, 2 * hb + j, :],
                                        pet[:, j, :],
                                        start=(pk == 0), stop=(pk == KTB - 1))
                            prev = (ktb, et)
                            uacc += ustep
                            while ui < len(units) and uacc >= ui + 1:
                                units[ui]()
                                ui += 1
                        pk, pet = prev
                        for j in range(2):
                            nc.tensor.matmul(
                                accs[j][0:HD + 1, :],
                                vp[:, pk, 2 * hb + j, :], pet[:, j, :],
                                start=(pk == 0), stop=(pk == KTB - 1))
                        for j in range(2):
                            zr = stgp.tile([1, 512], BF16, tag="zr",
                                           name="zr")
                            with nc.allow_low_precision(
                                    reason="softmax 1/Z; DVE mul"):
                                nc.vector.reciprocal(zr,
                                                     accs[j][HD:HD + 1, :])
                            zb = stgp.tile([64, 512], BF16, tag="zb",
                                           name="zb")
                            nc.gpsimd.partition_broadcast(zb, zr)
                            nc.vector.tensor_mul(
                                ctxnT[64 * j:64 * j + 64, hb,
                                      qc * 512:(qc + 1) * 512],
                                accs[j][0:HD, :], zb)
                while ui < len(units):
                    units[ui]()
                    ui += 1
                if carry_tail is not None:
                    carry_tail()
                pending = []

                if upto < 3:
                    nc.sync.dma_start(out=outT_p[:, 0, :],
                                      in_=ctxnT[:, 0, 0:SQ])
                    carry, carry_tail = [], None
                    continue

                carry, carry_tail = mk_carry(par)
                if it == sreps - 1:
                    for u in carry:
                        u()
                    carry_tail()
                    carry, carry_tail = [], None

    nc.compile()
    return nc


def shard_inputs(q, k, v, Wq, bq, Wk, bk, Wv, bv, Wo, bo):
    """Host-side sharding: per-core input dicts (head-sharded)."""
    import ml_dtypes
    bf16 = ml_dtypes.bfloat16
    scale = np.float32(1.0 / np.sqrt(HD))
    c32 = lambda a: np.ascontiguousarray(a, dtype=np.float32)
    cbf = lambda a: np.ascontiguousarray(np.asarray(a, dtype=np.float32),
                                         dtype=bf16)
    Wq32, Wk32, Wv32, Wo32 = c32(Wq) * scale, c32(Wk), c32(Wv), c32(Wo)
    bq32, bk32, bv32 = c32(bq) * scale, c32(bk), c32(bv)
    boT = np.ascontiguousarray(c32(bo).reshape(MB, 128).T)
    qT = [cbf(np.asarray(q[b], np.float32).T) for b in range(B)]
    kT = [cbf(np.asarray(k[b], np.float32).T) for b in range(B)]
    vT = [cbf(np.asarray(v[b], np.float32).T) for b in range(B)]
    in_maps = []
    for c in range(N_CORES):
        b, g = c // CPG, c % CPG
        fsl = slice(g * HL * HD, (g + 1) * HL * HD)
        bvp = np.broadcast_to(
            cbf(bv32[fsl]).reshape(1, HL, HD), (128, HL, HD))
        in_maps.append({
            "qT": qT[b], "kT": kT[b], "vT": vT[b],
            "Wq": cbf(Wq32[:, fsl]), "Wk": cbf(Wk32[:, fsl]),
            "Wv": cbf(Wv32[:, fsl]),
            "Wo": cbf(Wo32[fsl, :]),
            "bqT": np.ascontiguousarray(bq32[fsl].reshape(FB, 128).T),
            "bkT": np.ascontiguousarray(bk32[fsl].reshape(FB, 128).T),
            "boT": boT,
            "bvp": np.ascontiguousarray(bvp),
        })
    return in_maps


_NC_CACHE = {}


def get_nc():
    if "nc" not in _NC_CACHE:
        _NC_CACHE["nc"] = build_nc()
    return _NC_CACHE["nc"]


def kernel(q, k, v, Wq, bq, Wk, bk, Wv, bv, Wo, bo):
    q, k, v = np.asarray(q), np.asarray(k), np.asarray(v)
    in_maps = shard_inputs(q, k, v, Wq, bq, Wk, bk, Wv, bv, Wo, bo)
    nc = get_nc()
    res = run_bass_kernel_spmd(nc, in_maps, core_ids=list(range(N_CORES)))
    out = np.empty((B, S, H), dtype=np.float32)
    for c in range(N_CORES):
        b, r0 = c // CPG, (c % CPG) * SQ
        out[b, r0:r0 + SQ, :] = np.asarray(
            res.results[c]["outT"], dtype=np.float32).T
    return out


# revision 21
# speedup vs baseline: 1.2671x; 1.2417x over previous
"""Trainium2 Bass kernel for DeTrAttention -- HEAD-SHARDED variant.

Full op: out = softmax((q@Wq+bq)(k@Wk+bk)^T / sqrt(64)) (v@Wv+bv) @ Wo + bo
Shapes: q,k,v [B=2, S=2048, H=1024], NH=16 heads, HD=64.

Sharding (8 cores): data-parallel over batch (2 groups of 4 cores); within a
group, core g owns HEADS 4g..4g+3 end-to-end (Q/K/V projected only into its
256 feature columns, attention over ALL 2048 tokens) -- so no K/V exchange
is needed at all.  The output projection contracts each core's 256 context
features against its 256-row slice of Wo, giving a PARTIAL output for all
2048 tokens; a 4-rank bf16 ReduceScatter sums the partials and hands core g
its 512 token rows.  Unlike the K/V AllGather of the query-sharded variant
(which sat ~60us un-hidden on the critical path), the ReduceScatter has a
full attention phase to complete in and moves half the bytes.

Schedule: software-pipelined like the query-sharded variant: iteration
it-1's deferred output projection + ReduceScatter launch, it+1's input
loads + Q/K/V projections, and it-1's reduced-output writeback all
interleave into iteration it's attention steps (paced evenly).  Q/K/V
projections write kpT/qpT/vp SBUF directly (no DRAM staging); all four
weight matrices stay SBUF-resident (loaded once).  Scores use PE row
tiling: each head's QK^T has K=64 contraction, so head pairs run as
[64,128] stationaries at tile_position (0,0)/(64,0) concurrently.

Precision: bf16 weights/activations (Wq,bq pre-scaled by 1/sqrt(64)), fp32
PSUM accumulation, bf16 partials into the ReduceScatter (adds ~1e-3 noise,
well inside the 2e-2 gate).  exp(s - 2) with the ones-column Z trick as
before.
"""

import numpy as np

import concourse.bass as bass
import concourse.tile as tile
from concourse import bacc, mybir
from concourse.bass_utils import run_bass_kernel_spmd

F32 = mybir.dt.float32
BF16 = mybir.dt.bfloat16
U8 = mybir.dt.uint8

B, S, H, NH = 2, 2048, 1024, 16
HD = H // NH  # 64
N_CORES = 8
CPG = 4            # cores per batch group
HL = NH // CPG     # local heads (4)
FB = HL * HD // 128  # local feature blocks (2)
SQ = S // CPG      # output token rows owned per core (512)
KB = H // 128      # contraction 128-blocks (8)
MB = H // 128      # output-feature 128-blocks (8)
KTB = S // 128     # key-token 128-blocks (16)
QC = S // 512      # query 512-chunks (4)
EBIAS = -2.0       # exp(s + EBIAS); cancels in softmax
GROUPS = [[0, 1, 2, 3], [4, 5, 6, 7]]


def build_nc(sreps=1, upto=3, sim=False):
    """Per-core Bass program (SPMD, identical on all 8 cores).

    sim=True replaces the ReduceScatter with a local DMA copy (timing
    shape only; numerically wrong) so single-core TimelineSim works.
    """
    nc = bacc.Bacc("TRN2", target_bir_lowering=False, debug=False,
                   num_devices=8)

    qT = nc.dram_tensor("qT", [H, S], BF16, kind="ExternalInput").ap()
    kT = nc.dram_tensor("kT", [H, S], BF16, kind="ExternalInput").ap()
    vT = nc.dram_tensor("vT", [H, S], BF16, kind="ExternalInput").ap()
    Wq = nc.dram_tensor("Wq", [H, HL * HD], BF16, kind="ExternalInput").ap()
    Wk = nc.dram_tensor("Wk", [H, HL * HD], BF16, kind="ExternalInput").ap()
    Wv = nc.dram_tensor("Wv", [H, HL * HD], BF16, kind="ExternalInput").ap()
    Wo = nc.dram_tensor("Wo", [HL * HD, H], BF16, kind="ExternalInput").ap()
    bqT = nc.dram_tensor("bqT", [128, FB], F32, kind="ExternalInput").ap()
    bkT = nc.dram_tensor("bkT", [128, FB], F32, kind="ExternalInput").ap()
    boT = nc.dram_tensor("boT", [128, MB], F32, kind="ExternalInput").ap()
    # bv pre-broadcast to all partitions, [128, HL, HD] (no ones col needed;
    # the ones column of vp is memset once)
    bvp = nc.dram_tensor("bvp", [128, HL, HD], BF16,
                         kind="ExternalInput").ap()
    outT = nc.dram_tensor("outT", [H, SQ], BF16, kind="ExternalOutput").ap()

    qT_p = qT.rearrange("(kb p) t -> p kb t", p=128)
    kT_p = kT.rearrange("(kb p) t -> p kb t", p=128)
    vT_p = vT.rearrange("(kb p) t -> p kb t", p=128)
    Wq_p = Wq.rearrange("(kb p) o -> p kb o", p=128)
    Wk_p = Wk.rearrange("(kb p) o -> p kb o", p=128)
    Wv_p = Wv.rearrange("(kb p) o -> p kb o", p=128)
    Wo_p = Wo.rearrange("(fb p) o -> p fb o", p=128)
    outT_p = outT.rearrange("(ob p) t -> p ob t", p=128)

    with tile.TileContext(nc) as tc:
        with tc.tile_pool(name="consts", bufs=1) as consts, \
             tc.tile_pool(name="persist", bufs=1) as persist, \
             tc.tile_pool(name="stream", bufs=3) as stream, \
             tc.tile_pool(name="exps", bufs=3) as exps, \
             tc.tile_pool(name="stg", bufs=2) as stgp, \
             tc.tile_pool(name="ow", bufs=2) as owp, \
             tc.tile_pool(name="dramp", bufs=2, space="DRAM") as dramp, \
             tc.tile_pool(name="ps2b", bufs=2, space="PSUM") as ps2b, \
             tc.tile_pool(name="psa", bufs=2, space="PSUM") as psA, \
             tc.tile_pool(name="psacc", bufs=2, space="PSUM") as psacc:

            ebias = consts.tile([128, 1], F32)
            nc.vector.memset(ebias, EBIAS)
            bq_sb = consts.tile([128, FB], F32, tag="bq")
            bk_sb = consts.tile([128, FB], F32, tag="bk")
            bo_sb = consts.tile([128, MB], F32, tag="bo")
            bvp_sb = consts.tile([128, HL, HD], BF16, tag="bvp")
            nc.sync.dma_start(out=bq_sb, in_=bqT)
            nc.sync.dma_start(out=bk_sb, in_=bkT)
            nc.sync.dma_start(out=bo_sb, in_=boT)
            nc.sync.dma_start(out=bvp_sb, in_=bvp)
            # resident weights (loaded once)
            wq_sb = consts.tile([128, KB, HL * HD], BF16, tag="wq")
            wk_sb = consts.tile([128, KB, HL * HD], BF16, tag="wk")
            wv_sb = consts.tile([128, KB, HL * HD], BF16, tag="wv")
            wo_sb = consts.tile([128, FB, H], BF16, tag="wo")
            nc.sync.dma_start(out=wq_sb, in_=Wq_p)
            nc.sync.dma_start(out=wk_sb, in_=Wk_p)
            nc.sync.dma_start(out=wv_sb, in_=Wv_p)
            nc.sync.dma_start(out=wo_sb, in_=Wo_p)

            # double-buffered per-iteration state (parity = iteration % 2)
            kpTs = [persist.tile([128, FB, S], BF16, tag=f"kpT{x}",
                                 name=f"kpT{x}") for x in "AB"]
            qpTs = [persist.tile([128, FB, S], BF16, tag=f"qpT{x}",
                                 name=f"qpT{x}") for x in "AB"]
            vps = [persist.tile([128, KTB, HL, HD + 1], BF16, tag=f"vp{x}",
                                name=f"vp{x}") for x in "AB"]
            ctxnTs = [persist.tile([128, FB, S], BF16, tag=f"ctxnT{x}",
                                   name=f"ctxnT{x}") for x in "AB"]
            # ones column of vp written once; projections only touch [0:HD]
            for _v in vps:
                nc.vector.memset(_v[:, :, :, HD:HD + 1], 1.0)

            def emit_iter_inputs(it):
                """Unit closures for iteration `it`'s input pipeline:
                input loads + K/Q/V projections straight into SBUF."""
                par = it % 2
                st8 = {}
                ems = []

                def e_in(name, dram, half):
                    def u():
                        if half == 0:
                            t = stream.tile([128, KB, S], BF16, tag="in3",
                                            name=f"{name}_t")
                            st8[name] = t
                        nc.sync.dma_start(
                            out=st8[name][:, :, half * 1024:(half + 1) * 1024],
                            in_=dram[:, :, half * 1024:(half + 1) * 1024])
                    return u

                def e_kq(which, w_sb, dst, b_sb, fb, tc4):
                    def u():
                        ps = psA.tile([128, 512], F32, tag="psa",
                                      name=f"{which}p{fb}{tc4}")
                        src = st8[which]
                        for kb in range(KB):
                            nc.tensor.matmul(
                                ps, w_sb[:, kb, fb * 128:(fb + 1) * 128],
                                src[:, kb, tc4 * 512:(tc4 + 1) * 512],
                                start=(kb == 0), stop=(kb == KB - 1))
                        nc.vector.tensor_scalar_add(
                            dst[:, fb, tc4 * 512:(tc4 + 1) * 512], ps,
                            b_sb[:, fb:fb + 1])
                    return u

                def e_vp(st):
                    def u():
                        ps = psA.tile([128, HL * HD], F32, tag="psa",
                                      name=f"vp{st}")
                        for kb in range(KB):
                            nc.tensor.matmul(
                                ps, st8["v"][:, kb, st * 128:(st + 1) * 128],
                                wv_sb[:, kb, :],
                                start=(kb == 0), stop=(kb == KB - 1))
                        nc.vector.tensor_add(
                            vps[par][:, st, :, 0:HD],
                            ps.rearrange("p (hh d) -> p hh d", d=HD),
                            bvp_sb)
                    return u

                # all input loads FIRST (30+ steps of lead time): the PE
                # queue is strict FIFO, so a projection matmul whose input
                # DMA hasn't landed blocks every attention matmul behind
                # it and starves the scalar engine (stream bufs=3 lets the
                # three input tiles coexist without a pool-wait deadlock)
                for name, dram in (("k", kT_p), ("v", vT_p), ("q", qT_p)):
                    for half in range(2):
                        ems.append(e_in(name, dram, half))
                for fb in range(FB):
                    for tc4 in range(QC):
                        ems.append(e_kq("k", wk_sb, kpTs[par], bk_sb,
                                        fb, tc4))
                for st in range(KTB):
                    ems.append(e_vp(st))
                for fb in range(FB):
                    for tc4 in range(QC):
                        ems.append(e_kq("q", wq_sb, qpTs[par], bq_sb,
                                        fb, tc4))
                return ems, st8

            def mk_carry(par):
                """Deferred tail of iteration with parity `par`: output
                projection (staged dest-major to DRAM), ReduceScatter
                launch, and (last) the reduced-output writeback."""
                rs_in = dramp.tile([CPG, 128, MB, SQ], BF16, tag="rsi",
                                   name="rs_in")
                rs_out = dramp.tile([128, MB, SQ], BF16, tag="rso",
                                    name="rs_out")
                units = []

                def u_ob(ob):
                    for tc4 in range(QC):
                        po = psA.tile([128, 512], F32, tag="psa",
                                      name=f"po{ob}{tc4}")
                        for fb in range(FB):
                            nc.tensor.matmul(
                                po, wo_sb[:, fb, ob * 128:(ob + 1) * 128],
                                ctxnTs[par][:, fb,
                                            tc4 * 512:(tc4 + 1) * 512],
                                start=(fb == 0), stop=(fb == FB - 1))
                        pst = stgp.tile([128, SQ], BF16, tag="kst",
                                        name="pst")
                        nc.vector.tensor_copy(pst, po)
                        nc.sync.dma_start(out=rs_in[tc4, :, ob, :], in_=pst)
                for ob in range(MB):
                    units.append(lambda ob=ob: u_ob(ob))

                def u_rs():
                    if sim:
                        nc.sync.dma_start(out=rs_out, in_=rs_in[0])
                    else:
                        nc.gpsimd.collective_compute(
                            "ReduceScatter", mybir.AluOpType.add,
                            ins=[rs_in.opt()], outs=[rs_out.opt()],
                            replica_groups=GROUPS)
                units.append(u_rs)

                def u_write():
                    # SP queue, NOT gpsimd: a wait on the ReduceScatter at
                    # the head of the gpsimd FIFO would block the next
                    # attention's norm partition_broadcasts behind it
                    # (input loads queued on SP have far more slack).
                    ot = owp.tile([128, MB, SQ], BF16, tag="ot", name="ot")
                    nc.sync.dma_start(out=ot, in_=rs_out)
                    for ob in range(MB):
                        oo = stgp.tile([128, SQ], BF16, tag="kst",
                                       name="oo")
                        nc.vector.tensor_scalar_add(oo, ot[:, ob, :],
                                                    bo_sb[:, ob:ob + 1])
                        nc.sync.dma_start(out=outT_p[:, ob, :], in_=oo)
                return units, u_write

            pending, pstate = emit_iter_inputs(0)
            carry = []        # prev iteration's outproj + RS units
            carry_tail = None  # prev iteration's writeback unit
            for it in range(sreps):
                par = it % 2
                kpT, vp, qpT, ctxnT = (kpTs[par], vps[par], qpTs[par],
                                       ctxnTs[par])
                for e in pending:
                    e()
                pending = []

                front = []
                if it + 1 < sreps:
                    front, pstate = emit_iter_inputs(it + 1)
                pending = front  # for the non-attention paths

                if upto < 2:
                    nc.sync.dma_start(out=outT_p[:, 0, :],
                                      in_=kpT[:, 0, 0:SQ])
                    nc.sync.dma_start(out=outT_p[:, 1, :],
                                      in_=qpT[:, 0, 0:SQ])
                    nc.gpsimd.dma_start(out=outT_p[:, 2, 0:65],
                                        in_=vp[:, 0, 0, :])
                    carry, carry_tail = [], None
                    continue

                # ---- attention: 128 steps of (row-tiled scores pair, exp,
                # lag-1 ctx pair).  carry units (it-1's outproj + RS
                # launch) go 1-per-step so the ReduceScatter starts as
                # early as possible; front units (it+1's inputs +
                # projections) spread over the remaining steps; the
                # writeback of it-1 goes at the tail ----
                units = carry + front
                n_steps = FB * QC * KTB
                ustep = len(units) / n_steps
                uacc = 0.0
                ui = 0
                for hb in range(FB):
                    for qc in range(QC):
                        accs = [psacc.tile([128, 512], F32, tag="acc",
                                           name=f"acc{j}") for j in range(2)]
                        prev = None
                        for ktb in range(KTB):
                            p1 = ps2b.tile([128, 2, 512], F32, tag="sc",
                                           name="sc")
                            for j in range(2):
                                nc.tensor.matmul(
                                    p1[:, j, :],
                                    kpT[64 * j:64 * j + 64, hb,
                                        ktb * 128:(ktb + 1) * 128],
                                    qpT[64 * j:64 * j + 64, hb,
                                        qc * 512:(qc + 1) * 512],
                                    start=True, stop=True)
                            et = exps.tile([128, 2, 512], BF16, tag="et",
                                           name="et")
                            nc.scalar.activation(
                                out=et, in_=p1,
                                func=mybir.ActivationFunctionType.Exp,
                                bias=ebias[:, 0:1])
                            if prev is not None:
                                pk, pet = prev
                                for j in range(2):
                                    nc.tensor.matmul(
                                        accs[j][0:HD + 1, :],
                                        vp[:, pk, 2 * hb + j, :],
                                        pet[:, j, :],
                                        start=(pk == 0), stop=(pk == KTB - 1))
                            prev = (ktb, et)
                            uacc += ustep
                            while ui < len(units) and uacc >= ui + 1:
                                units[ui]()
                                ui += 1
                        pk, pet = prev
                        for j in range(2):
                            nc.tensor.matmul(
                                accs[j][0:HD + 1, :],
                                vp[:, pk, 2 * hb + j, :], pet[:, j, :],
                                start=(pk == 0), stop=(pk == KTB - 1))
                        for j in range(2):
                            zr = stgp.tile([1, 512], BF16, tag="zr",
                                           name="zr")
                            with nc.allow_low_precision(
                                    reason="softmax 1/Z; DVE mul"):
                                nc.vector.reciprocal(zr,
                                                     accs[j][HD:HD + 1, :])
                            zb = stgp.tile([64, 512], BF16, tag="zb",
                                           name="zb")
                            nc.gpsimd.partition_broadcast(zb, zr)
                            nc.vector.tensor_mul(
                                ctxnT[64 * j:64 * j + 64, hb,
                                      qc * 512:(qc + 1) * 512],
                                accs[j][0:HD, :], zb)
                while ui < len(units):
                    units[ui]()
                    ui += 1
                if carry_tail is not None:
                    carry_tail()
                pending = []

                if upto < 3:
                    nc.sync.dma_start(out=outT_p[:, 0, :],
                                      in_=ctxnT[:, 0, 0:SQ])
                    carry, carry_tail = [], None
                    continue

                carry, carry_tail = mk_carry(par)
                if it == sreps - 1:
                    for u in carry:
                        u()
                    carry_tail()
                    carry, carry_tail = [], None

    nc.compile()
    return nc


def shard_inputs(q, k, v, Wq, bq, Wk, bk, Wv, bv, Wo, bo):
    """Host-side sharding: per-core input dicts (head-sharded)."""
    import ml_dtypes
    bf16 = ml_dtypes.bfloat16
    scale = np.float32(1.0 / np.sqrt(HD))
    c32 = lambda a: np.ascontiguousarray(a, dtype=np.float32)
    cbf = lambda a: np.ascontiguousarray(np.asarray(a, dtype=np.float32),
                                         dtype=bf16)
    Wq32, Wk32, Wv32, Wo32 = c32(Wq) * scale, c32(Wk), c32(Wv), c32(Wo)
    bq32, bk32, bv32 = c32(bq) * scale, c32(bk), c32(bv)
    boT = np.ascontiguousarray(c32(bo).reshape(MB, 128).T)
    qT = [cbf(np.asarray(q[b], np.float32).T) for b in range(B)]
    kT = [cbf(np.asarray(k[b], np.float32).T) for b in range(B)]
    vT = [cbf(np.asarray(v[b], np.float32).T) for b in range(B)]
    in_maps = []
    for c in range(N_CORES):
        b, g = c // CPG, c % CPG
        fsl = slice(g * HL * HD, (g + 1) * HL * HD)
        bvp = np.broadcast_to(
            cbf(bv32[fsl]).reshape(1, HL, HD), (128, HL, HD))
        in_maps.append({
            "qT": qT[b], "kT": kT[b], "vT": vT[b],
            "Wq": cbf(Wq32[:, fsl]), "Wk": cbf(Wk32[:, fsl]),
            "Wv": cbf(Wv32[:, fsl]),
            "Wo": cbf(Wo32[fsl, :]),
            "bqT": np.ascontiguousarray(bq32[fsl].reshape(FB, 128).T),
            "bkT": np.ascontiguousarray(bk32[fsl].reshape(FB, 128).T),
            "boT": boT,
            "bvp": np.ascontiguousarray(bvp),
        })
    return in_maps


_NC_CACHE = {}


def get_nc():
    if "nc" not in _NC_CACHE:
        _NC_CACHE["nc"] = build_nc()
    return _NC_CACHE["nc"]


def kernel(q, k, v, Wq, bq, Wk, bk, Wv, bv, Wo, bo):
    q, k, v = np.asarray(q), np.asarray(k), np.asarray(v)
    in_maps = shard_inputs(q, k, v, Wq, bq, Wk, bk, Wv, bv, Wo, bo)
    nc = get_nc()
    res = run_bass_kernel_spmd(nc, in_maps, core_ids=list(range(N_CORES)))
    out = np.empty((B, S, H), dtype=np.float32)
    for c in range(N_CORES):
        b, r0 = c // CPG, (c % CPG) * SQ
        out[b, r0:r0 + SQ, :] = np.asarray(
            res.results[c]["outT"], dtype=np.float32).T
    return out
